# revision 1
# baseline (speedup 1.0000x reference)
"""Trainium2 Bass kernel for a dense transformer block (nn_Block_30262339567972).

Full inputs in, full outputs out. Internally sharded across 8 NeuronCores with
zero collectives: core c = 2*b + j owns two 512-token chunks of batch b
(j=0 -> chunks {0,3}, j=1 -> chunks {1,2}; the pairing balances causal
attention work). Each core computes LN1 and K/V for the whole 2048-token
sequence itself, Q/attention/proj/MLP only for its own 1024 tokens, and
writes its tokens' final output. The host concatenates.

Everything on device is feature-major (activations [feature, token]); the
host supplies x pre-transposed and transposes the output back. LayerNorm
statistics are computed with ones-vector matmuls on the PE (partition
reductions), so no on-device transposes exist at all. Matmuls run in
float32r (single-pass fp32, ~1.5e-4 rms error per matmul on HW). Attention
scores are produced in [k, q] layout where an appended ones-column on V
yields softmax denominators for free; probabilities stay unnormalized until
a per-head reciprocal broadcast at the end.
"""

from contextlib import ExitStack

import numpy as np

import concourse.bacc as bacc
import concourse.bass as bass
import concourse.tile as tile
from concourse import mybir
from concourse.bass_utils import run_bass_kernel_spmd
import concourse.bass_utils as _bu

if not getattr(_bu, "_ldw_opt_patched", False):
    _orig_run_command = _bu.run_command

    def _run_command_ldw(argv, **kw):
        argv = ["--enable-ldw-opt=true" if a == "--enable-ldw-opt=false" else a
                for a in argv]
        return _orig_run_command(argv, **kw)

    _bu.run_command = _run_command_ldw
    _bu._ldw_opt_patched = True

F32 = mybir.dt.float32
F32R = mybir.dt.float32r
P = 128
B, T, C = 4, 2048, 1024
H, D = 16, 64
DFF = 4096
TOWN = 1024            # tokens owned per core
NQC = TOWN // 512      # 2 query chunks of 512
EPS = 1e-5
SCALE = D ** -0.5
NEG = -1e30

KT_C = C // P          # 8 contraction tiles over C
FT_C = C // P          # 8 feature tiles over C
TT_FULL = T // P       # 16 token tiles (full seq)
TT_OWN = TOWN // P     # 8 token tiles (own)
NGROUP = H // 2        # 8 head-pair groups
NB_OWN = TOWN // 512   # 2 moving blocks over own tokens
NB_FULL = T // 512     # 4 moving blocks over full seq

Ident = mybir.ActivationFunctionType.Identity
Sqrt = mybir.ActivationFunctionType.Sqrt
Exp = mybir.ActivationFunctionType.Exp
Relu = mybir.ActivationFunctionType.Relu
ADD = mybir.AluOpType.add
SUB = mybir.AluOpType.subtract
MULT = mybir.AluOpType.mult


def _alloc(pool, n, shape, dt, prefix, **kw):
    return [
        pool.tile(list(shape), dt, tag=f"{prefix}{i}", name=f"{prefix}{i}", **kw)
        for i in range(n)
    ]


def _ln_feature_major(nc, tc, ctx, x_loader, dst_hT, ncols, g_col, b_col,
                      eps_t, ones1, st_ps, rowp, bcp, prefix):
    """LayerNorm in feature-major layout.

    x_loader(kt, nb) -> [P, 512] f32r AP for that block (may DMA into a
    transient tile). dst_hT: FT_C tiles (f32r out). Stats per 512-token
    block via ones-matmul partition reductions; mean/rstd rows broadcast
    across partitions with gpsimd; apply = DVE sub/mul then ACT
    per-partition gamma/beta."""
    for nb in range(ncols // 512):
        sl = slice(nb * 512, (nb + 1) * 512)
        xT_blk = [x_loader(kt, nb) for kt in range(KT_C)]
        ssum = st_ps.tile([1, 512], F32, tag="ssum", name=f"{prefix}ss{nb}")
        ssq = st_ps.tile([1, 512], F32, tag="ssq", name=f"{prefix}sq{nb}")
        for kt in range(KT_C):
            nc.tensor.matmul(ssum, ones1, xT_blk[kt],
                             start=(kt == 0), stop=(kt == KT_C - 1))
        for kt in range(KT_C):
            sq = rowp.tile([P, 512], F32R, tag="sqt", name=f"{prefix}sqt{nb}_{kt}")
            nc.vector.tensor_mul(out=sq, in0=xT_blk[kt], in1=xT_blk[kt])
            nc.tensor.matmul(ssq, ones1, sq,
                             start=(kt == 0), stop=(kt == KT_C - 1))
        mu = rowp.tile([1, 512], F32, tag="mu", name=f"{prefix}mu{nb}")
        nc.scalar.mul(mu, ssum, 1.0 / C)
        msq = rowp.tile([1, 512], F32, tag="msq", name=f"{prefix}msq{nb}")
        nc.scalar.mul(msq, ssq, 1.0 / C)
        var = rowp.tile([1, 512], F32, tag="var", name=f"{prefix}var{nb}")
        nc.vector.tensor_mul(out=var, in0=mu, in1=mu)
        nc.vector.tensor_sub(out=var, in0=msq, in1=var)
        std = rowp.tile([1, 512], F32, tag="std", name=f"{prefix}std{nb}")
        nc.scalar.activation(out=std, in_=var, func=Sqrt,
                             bias=eps_t[0:1, 0:1], scale=1.0)
        rs = rowp.tile([1, 512], F32, tag="rs", name=f"{prefix}rs{nb}")
        nc.vector.reciprocal(out=rs, in_=std)
        mu_b = bcp.tile([P, 512], F32, tag="mub", name=f"{prefix}mub{nb}")
        nc.gpsimd.partition_broadcast(mu_b, mu)
        rs_b = bcp.tile([P, 512], F32, tag="rsb", name=f"{prefix}rsb{nb}")
        nc.gpsimd.partition_broadcast(rs_b, rs)
        for ft in range(FT_C):
            t = rowp.tile([P, 512], F32, tag="ap", name=f"{prefix}ap{nb}_{ft}")
            nc.vector.tensor_sub(out=t, in0=xT_blk[ft].bitcast(F32),
                                 in1=mu_b)
            nc.vector.tensor_mul(out=t, in0=t, in1=rs_b)
            nc.scalar.activation(out=dst_hT[ft][:, sl], in_=t, func=Ident,
                                 bias=b_col[:, ft:ft + 1],
                                 scale=g_col[:, ft:ft + 1])


def build_nc():
    nc = bacc.Bacc()
    xT_full = nc.declare_dram_parameter("xT_full", [C, T], F32, isOutput=False)
    xT_own = nc.declare_dram_parameter("xT_own", [C, TOWN], F32, isOutput=False)
    mask_lo = nc.declare_dram_parameter("mask_lo", [512, 1024], F32, isOutput=False)
    mask_hi = nc.declare_dram_parameter("mask_hi", [512, 1024], F32, isOutput=False)
    attn_w = nc.declare_dram_parameter("attn_w", [C, 3 * C], F32, isOutput=False)
    attn_b = nc.declare_dram_parameter("attn_b", [3 * C], F32, isOutput=False)
    proj_w = nc.declare_dram_parameter("proj_w", [C, C], F32, isOutput=False)
    proj_b = nc.declare_dram_parameter("proj_b", [C], F32, isOutput=False)
    ln1_g = nc.declare_dram_parameter("ln1_g", [C], F32, isOutput=False)
    ln1_b = nc.declare_dram_parameter("ln1_b", [C], F32, isOutput=False)
    ln2_g = nc.declare_dram_parameter("ln2_g", [C], F32, isOutput=False)
    ln2_b = nc.declare_dram_parameter("ln2_b", [C], F32, isOutput=False)
    fc1_w = nc.declare_dram_parameter("fc1_w", [C, DFF], F32, isOutput=False)
    fc1_b = nc.declare_dram_parameter("fc1_b", [DFF], F32, isOutput=False)
    fc2_w = nc.declare_dram_parameter("fc2_w", [DFF, C], F32, isOutput=False)
    fc2_b = nc.declare_dram_parameter("fc2_b", [C], F32, isOutput=False)
    out = nc.declare_dram_parameter("out", [C, TOWN], F32, isOutput=True)

    # DRAM spill for K^T and V between the projection and attention phases.
    kT_dram = nc.dram_tensor("kT_dram", [NGROUP, P, T], F32R)
    v_dram = nc.dram_tensor("v_dram", [NGROUP, TT_FULL, P, 130], F32R)

    with tile.TileContext(nc) as tc, ExitStack() as top:
        const = top.enter_context(tc.tile_pool(name="const", bufs=1))
        eps_t = const.tile([P, 1], F32, name="eps_t")
        nc.vector.memset(eps_t, EPS)
        ones1f = const.tile([P, 1], F32, name="ones1f")
        nc.vector.memset(ones1f, 1.0)
        ones1 = const.tile([P, 1], F32R, name="ones1")
        nc.vector.tensor_copy(out=ones1, in_=ones1f)
        ones16 = const.tile([P, H], F32, name="ones16")
        nc.vector.memset(ones16, 1.0)
        ln1g_t = const.tile([P, FT_C], F32, name="ln1g_t")
        ln1b_t = const.tile([P, FT_C], F32, name="ln1b_t")
        ln2g_t = const.tile([P, FT_C], F32, name="ln2g_t")
        ln2b_t = const.tile([P, FT_C], F32, name="ln2b_t")
        nc.sync.dma_start(out=ln1g_t, in_=ln1_g.rearrange("(f p) -> p f", p=P))
        nc.sync.dma_start(out=ln1b_t, in_=ln1_b.rearrange("(f p) -> p f", p=P))
        nc.sync.dma_start(out=ln2g_t, in_=ln2_g.rearrange("(f p) -> p f", p=P))
        nc.sync.dma_start(out=ln2b_t, in_=ln2_b.rearrange("(f p) -> p f", p=P))
        abq_t = const.tile([P, NGROUP], F32, name="abq_t")
        abk_t = const.tile([P, NGROUP], F32, name="abk_t")
        nc.sync.dma_start(out=abq_t, in_=attn_b[0:C].rearrange("(g p) -> p g", p=P))
        nc.sync.dma_start(out=abk_t,
                          in_=attn_b[C:2 * C].rearrange("(g p) -> p g", p=P))
        projb_t = const.tile([P, FT_C], F32, name="projb_t")
        nc.sync.dma_start(out=projb_t, in_=proj_b.rearrange("(f p) -> p f", p=P))
        fc2b_t = const.tile([P, FT_C], F32, name="fc2b_t")
        nc.sync.dma_start(out=fc2b_t, in_=fc2_b.rearrange("(f p) -> p f", p=P))
        fc1b_t = const.tile([P, DFF // P], F32, name="fc1b_t")
        nc.sync.dma_start(out=fc1b_t, in_=fc1_b.rearrange("(f p) -> p f", p=P))

        # BIG pool: slot classes recycled across phases (same tag = same slot):
        #   Y: qT (P1-P3) -> x2T (P4-P6)
        #   Z: attnT (P3-P4) -> mlpT (P5-P6)
        big = top.enter_context(tc.tile_pool(name="big", bufs=1))

        # ---- Phase 1a: LN1(own) -> hT_own; Q^T (Y) ----
        with ExitStack() as c1:
            hTo_pool = c1.enter_context(tc.tile_pool(name="hTo_pool", bufs=1))
            hT_own = _alloc(hTo_pool, FT_C, [P, TOWN], F32R, "hTo")
            with ExitStack() as c1a:
                st_ps = c1a.enter_context(tc.tile_pool(name="st_ps", bufs=2,
                                                       space="PSUM"))
                rowp = c1a.enter_context(tc.tile_pool(name="rowp", bufs=3))
                bcp = c1a.enter_context(tc.tile_pool(name="bcp", bufs=2))
                lnp = c1a.enter_context(tc.tile_pool(name="lnp", bufs=1))

                def own_loader(kt, nb):
                    t = lnp.tile([P, 512], F32R, tag=f"xo{kt}",
                                 name=f"xo{kt}_{nb}", bufs=1)
                    nc.sync.dma_start(
                        out=t,
                        in_=xT_own[kt * P:(kt + 1) * P,
                                   nb * 512:(nb + 1) * 512].bitcast(F32R))
                    return t[:, :]
                _ln_feature_major(nc, tc, c1a, own_loader, hT_own, TOWN,
                                  ln1g_t, ln1b_t, eps_t, ones1, st_ps, rowp,
                                  bcp, "lo")

            qT = [big.tile([P, TOWN], F32R, tag=f"Y{i}", name=f"qT{i}")
                  for i in range(NGROUP)]
            with ExitStack() as c2:
                wstream = c2.enter_context(tc.tile_pool(name="wstream", bufs=1))
                mm_ps = c2.enter_context(
                    tc.tile_pool(name="mm_ps", bufs=2, space="PSUM"))

                def stream_w(dram_slice, tag, name, ncols, bufs=2):
                    w = wstream.tile([P, ncols], F32R, tag=tag,
                                     name=f"{name}_w", bufs=bufs)
                    nc.sync.dma_start(out=w, in_=dram_slice.bitcast(F32R))
                    return w

                for g in range(NGROUP):
                    wq_g = [stream_w(
                        attn_w[kt * P:(kt + 1) * P, g * P:(g + 1) * P],
                        f"wq{kt}", f"wq{g}_{kt}", P) for kt in range(KT_C)]
                    pss = [mm_ps.tile([P, 512], F32, tag=f"mm{nb}",
                                      name=f"qps{g}_{nb}")
                           for nb in range(NB_OWN)]
                    for kt in range(KT_C):
                        for nb in range(NB_OWN):
                            nc.tensor.matmul(
                                pss[nb], wq_g[kt],
                                hT_own[kt][:, nb * 512:(nb + 1) * 512],
                                start=(kt == 0), stop=(kt == KT_C - 1))
                    for nb in range(NB_OWN):
                        nc.vector.tensor_scalar_add(
                            out=qT[g][:, nb * 512:(nb + 1) * 512], in0=pss[nb],
                            scalar1=abq_t[:, g:g + 1])

        # ---- Phase 2: per half of the full sequence: LN1 -> hT,
        #      then V rows and K^T columns for that half ----
        with ExitStack() as c2:
            wstream = c2.enter_context(tc.tile_pool(name="wstream2", bufs=1))
            mm_ps = c2.enter_context(
                tc.tile_pool(name="mm_ps2", bufs=2, space="PSUM"))

            def stream_w(dram_slice, tag, name, ncols, bufs=2):
                w = wstream.tile([P, ncols], F32R, tag=tag,
                                 name=f"{name}_w", bufs=bufs)
                nc.sync.dma_start(out=w, in_=dram_slice.bitcast(F32R))
                return w

            if True:
                for half in range(2):
                    with ExitStack() as ch:
                        hfp = ch.enter_context(
                            tc.tile_pool(name=f"hfp{half}", bufs=1))
                        hT = [hfp.tile([P, TOWN], F32R, tag=f"hf{i}",
                                       name=f"hTf{half}_{i}")
                              for i in range(FT_C)]
                        with ExitStack() as cl:
                            st2 = cl.enter_context(
                                tc.tile_pool(name=f"st2_{half}", bufs=2,
                                             space="PSUM"))
                            rowp2 = cl.enter_context(
                                tc.tile_pool(name=f"rowp2_{half}", bufs=3))
                            bcp2 = cl.enter_context(
                                tc.tile_pool(name=f"bcp2_{half}", bufs=1))
                            lnp2 = cl.enter_context(
                                tc.tile_pool(name=f"lnp2_{half}", bufs=1))
                            def full_loader(kt, nb, _h=half):
                                t = lnp2.tile([P, 512], F32R, tag=f"xf{kt}",
                                              name=f"xf{_h}_{kt}_{nb}",
                                              bufs=1)
                                nc.sync.dma_start(
                                    out=t,
                                    in_=xT_full[kt * P:(kt + 1) * P,
                                                _h * TOWN + nb * 512:
                                                _h * TOWN + (nb + 1) * 512
                                                ].bitcast(F32R))
                                return t[:, :]
                            _ln_feature_major(nc, tc, cl, full_loader, hT,
                                              TOWN, ln1g_t, ln1b_t, eps_t,
                                              ones1, st2, rowp2, bcp2,
                                              f"lf{half}")

                        with ExitStack() as cs2:
                            spill = cs2.enter_context(
                                tc.tile_pool(name=f"spill{half}", bufs=2))
                            vspill = cs2.enter_context(
                                tc.tile_pool(name=f"vspill{half}", bufs=3))
                            bvp = cs2.enter_context(
                                tc.tile_pool(name=f"bvp{half}", bufs=1))
                            bv_bc = bvp.tile([P, C], F32, name=f"bv_bc{half}")
                            abv = attn_b[2 * C:3 * C]
                            nc.sync.dma_start(
                                out=bv_bc,
                                in_=bass.AP(tensor=abv.tensor,
                                            offset=abv.offset,
                                            ap=[[0, P]] + list(abv.ap[-1:])))

                            # V (token-major, +bias, ones col)
                            wv_all = [[stream_w(
                                attn_w[kt * P:(kt + 1) * P,
                                       2 * C + nb * 512:
                                       2 * C + (nb + 1) * 512],
                                f"wv{kt}_{nb}", f"wv{half}_{kt}_{nb}", 512,
                                bufs=1) for nb in range(2)]
                                for kt in range(KT_C)]
                            for tt in range(TT_OWN):
                                gt = half * TT_OWN + tt
                                vt = vspill.tile(
                                    [P, H, 65], F32R, tag="vsp",
                                    name=f"vsp{half}_{tt}")
                                pss = [mm_ps.tile(
                                    [P, 512], F32, tag=f"mm{nb}",
                                    name=f"vps{half}_{tt}_{nb}")
                                    for nb in range(2)]
                                for kt in range(KT_C):
                                    for nb in range(2):
                                        nc.tensor.matmul(
                                            pss[nb],
                                            hT[kt][:, tt * P:(tt + 1) * P],
                                            wv_all[kt][nb], start=(kt == 0),
                                            stop=(kt == KT_C - 1))
                                for nb in range(2):
                                    nc.vector.tensor_add(
                                        out=vt[:, nb * 8:(nb + 1) * 8, 0:64],
                                        in0=pss[nb].rearrange(
                                            "p (h d) -> p h d", d=64),
                                        in1=bv_bc[:, nb * 512:(nb + 1) * 512]
                                        .rearrange("p (h d) -> p h d", d=64))
                                nc.vector.tensor_copy(
                                    out=vt[:, :, 64:65],
                                    in_=ones16.rearrange(
                                        "p (h o) -> p h o", o=1))
                                nc.sync.dma_start(
                                    out=v_dram[:, gt].rearrange(
                                        "g p x -> p g x"),
                                    in_=vt.rearrange(
                                        "p (g h) d -> p g (h d)", h=2))

                            # K^T columns for this half -> DRAM
                            for g in range(NGROUP):
                                wk_g = [stream_w(
                                    attn_w[kt * P:(kt + 1) * P,
                                           C + g * P:C + (g + 1) * P],
                                    f"wk{kt}", f"wk{half}_{g}_{kt}", P)
                                    for kt in range(KT_C)]
                                ksp = spill.tile([P, TOWN], F32R, tag="ksp",
                                                 name=f"ksp{half}_{g}")
                                pss = [mm_ps.tile(
                                    [P, 512], F32, tag=f"mm{nb}",
                                    name=f"kps{half}_{g}_{nb}")
                                    for nb in range(NB_OWN)]
                                for kt in range(KT_C):
                                    for nb in range(NB_OWN):
                                        nc.tensor.matmul(
                                            pss[nb], wk_g[kt],
                                            hT[kt][:, nb * 512:(nb + 1) * 512],
                                            start=(kt == 0),
                                            stop=(kt == KT_C - 1))
                                for nb in range(NB_OWN):
                                    nc.vector.tensor_scalar_add(
                                        out=ksp[:, nb * 512:(nb + 1) * 512],
                                        in0=pss[nb],
                                        scalar1=abk_t[:, g:g + 1])
                                nc.sync.dma_start(
                                    out=kT_dram[g][:, half * TOWN:
                                                   (half + 1) * TOWN],
                                    in_=ksp)

        # ---- Phase 3: attention per head-pair group ----
        attnT = [big.tile([P, TOWN], F32R, tag=f"Z{i}", name=f"attnT{i}")
                 for i in range(FT_C)]
        with ExitStack() as c3:
            mpool = c3.enter_context(tc.tile_pool(name="mpool", bufs=1))
            mlo = _alloc(mpool, 4, [P, 1024], F32, "mlo")
            mhi = _alloc(mpool, 4, [P, 1024], F32, "mhi")
            for k2 in range(4):
                nc.sync.dma_start(out=mlo[k2],
                                  in_=mask_lo[k2 * P:(k2 + 1) * P, :])
                nc.sync.dma_start(out=mhi[k2],
                                  in_=mask_hi[k2 * P:(k2 + 1) * P, :])

            gstream = c3.enter_context(tc.tile_pool(name="gstream", bufs=2))
            sc_ps = c3.enter_context(
                tc.tile_pool(name="sc_ps", bufs=2, space="PSUM"))
            y_ps_pool = c3.enter_context(
                tc.tile_pool(name="y_ps_pool", bufs=1, space="PSUM"))
            ppool = c3.enter_context(tc.tile_pool(name="ppool", bufs=4))
            npool = c3.enter_context(tc.tile_pool(name="npool", bufs=4))

            for g in range(NGROUP):
                kT_g = gstream.tile([P, T], F32R, tag="ktg", name=f"ktg{g}")
                nc.sync.dma_start(out=kT_g, in_=kT_dram[g])
                v_g = gstream.tile([P, TT_FULL, 130], F32R, tag="vg",
                                   name=f"vg{g}")
                nc.sync.dma_start(
                    out=v_g, in_=v_dram[g].rearrange("tt p x -> p tt x"))
                # merged query-chunk loop: k/v weight tiles feed both
                # chunks back-to-back so walrus ldw-opt elides the reload.
                y_ps = {
                    (qc, hh): y_ps_pool.tile([65, 512], F32,
                                             tag=f"y{qc}{hh}",
                                             name=f"y{g}_{qc}_{hh}")
                    for qc in range(NQC) for hh in range(2)
                }
                for k2 in range(8):
                    for hh in range(2):
                        hsl = slice(64 * hh, 64 * (hh + 1))
                        scs = {}
                        if k2 < 4:
                            scs[0] = sc_ps.tile([P, 1024], F32, tag="sc",
                                                name=f"sc{g}_0_{k2}_{hh}")
                        scs[1] = sc_ps.tile([P, 1024], F32, tag="sc",
                                            name=f"sc{g}_1_{k2}_{hh}")
                        for j in range(2):
                            kt = 2 * k2 + j
                            ksl = kT_g[hsl, kt * P:(kt + 1) * P]
                            for qc in scs:
                                nc.tensor.matmul(
                                    scs[qc][:, j * 512:(j + 1) * 512],
                                    ksl,
                                    qT[g][hsl, qc * 512:(qc + 1) * 512],
                                    start=True, stop=True,
                                    tile_position=(64 * hh, 0))
                        if k2 < 4:
                            nc.vector.tensor_add(out=scs[0], in0=scs[0],
                                                 in1=mlo[k2])
                        else:
                            nc.vector.tensor_add(out=scs[1], in0=scs[1],
                                                 in1=mhi[k2 - 4])
                        pts = {}
                        for qc in scs:
                            pts[qc] = ppool.tile([P, 1024], F32R, tag="pt",
                                                 name=f"p{g}_{qc}_{k2}_{hh}")
                            nc.scalar.activation(out=pts[qc], in_=scs[qc],
                                                 func=Exp, scale=SCALE)
                        for j in range(2):
                            kt = 2 * k2 + j
                            vsl = v_g[:, kt, 65 * hh:65 * (hh + 1)]
                            for qc in pts:
                                nc.tensor.matmul(
                                    y_ps[(qc, hh)],
                                    vsl,
                                    pts[qc][:, j * 512:(j + 1) * 512],
                                    start=(kt == 0),
                                    stop=(kt == (7 if qc == 0 else 15)))
                for qc in range(NQC):
                    for hh in range(2):
                        r = npool.tile([1, 512], F32, tag="r",
                                       name=f"r{g}_{qc}_{hh}")
                        nc.vector.reciprocal(out=r,
                                             in_=y_ps[(qc, hh)][64:65, :])
                        rb = npool.tile([64, 512], F32, tag="rb",
                                        name=f"rb{g}_{qc}_{hh}")
                        nc.gpsimd.partition_broadcast(rb, r[0:1, :])
                        nc.vector.tensor_mul(
                            out=attnT[g][64 * hh:64 * (hh + 1),
                                         qc * 512:(qc + 1) * 512],
                            in0=y_ps[(qc, hh)][0:64, :], in1=rb)

        # ---- Phase 4: proj (feature-major) + residual + LN2 ----
        x2T = [big.tile([P, TOWN], F32R, tag=f"Y{i}", name=f"x2T{i}")
               for i in range(FT_C)]
        s45 = ExitStack()
        h2T_pool = s45.enter_context(tc.tile_pool(name="h2T_pool", bufs=1))
        h2T = _alloc(h2T_pool, FT_C, [P, TOWN], F32R, "h2T")
        with ExitStack() as c4:
            w4 = c4.enter_context(tc.tile_pool(name="w4", bufs=1))
            pw = _alloc(w4, KT_C, [P, C], F32R, "pw")
            for kt in range(KT_C):
                nc.sync.dma_start(out=pw[kt],
                                  in_=proj_w[kt * P:(kt + 1) * P, :].bitcast(F32R))
            xop = c4.enter_context(tc.tile_pool(name="xop", bufs=3))
            mm_ps4 = c4.enter_context(
                tc.tile_pool(name="mm_ps4", bufs=2, space="PSUM"))

            for ft in range(FT_C):
                xo = xop.tile([P, TOWN], F32, tag="xo", name=f"xo{ft}")
                nc.sync.dma_start(out=xo, in_=xT_own[ft * P:(ft + 1) * P, :])
                pss = [mm_ps4.tile([P, 512], F32, tag=f"mm{nb}",
                                   name=f"prj{ft}_{nb}")
                       for nb in range(NB_OWN)]
                for kt in range(KT_C):
                    for nb in range(NB_OWN):
                        nc.tensor.matmul(
                            pss[nb], pw[kt][:, ft * P:(ft + 1) * P],
                            attnT[kt][:, nb * 512:(nb + 1) * 512],
                            start=(kt == 0), stop=(kt == KT_C - 1))
                for nb in range(NB_OWN):
                    sl = slice(nb * 512, (nb + 1) * 512)
                    t = xop.tile([P, 512], F32, tag="t4", name=f"t4{ft}_{nb}")
                    nc.vector.tensor_scalar_add(out=t, in0=pss[nb],
                                                scalar1=projb_t[:, ft:ft + 1])
                    nc.vector.tensor_add(out=x2T[ft][:, sl], in0=t,
                                         in1=xo[:, sl])

            st4 = c4.enter_context(tc.tile_pool(name="st4", bufs=2,
                                                space="PSUM"))
            rowp4 = c4.enter_context(tc.tile_pool(name="rowp4", bufs=3))
            bcp4 = c4.enter_context(tc.tile_pool(name="bcp4", bufs=2))
            _ln_feature_major(nc, tc, c4,
                              lambda kt, nb: x2T[kt][:, nb * 512:(nb + 1) * 512],
                              h2T, TOWN, ln2g_t, ln2b_t, eps_t, ones1, st4,
                              rowp4, bcp4, "l2")

        # ---- Phase 5: MLP (chunks of 512 over d_ff), mlpT feature-major ----
        mlpT = [big.tile([P, TOWN], F32, tag=f"Z{i}", name=f"mlpT{i}")
                for i in range(FT_C)]
        CH = 512           # d_ff chunk
        NM8 = CH // P      # 4 feature tiles per chunk
        with ExitStack() as c5:
            w5 = c5.enter_context(tc.tile_pool(name="w5", bufs=1))
            h1_pool = c5.enter_context(tc.tile_pool(name="h1_pool", bufs=1))
            mm_ps5 = c5.enter_context(
                tc.tile_pool(name="mm_ps5", bufs=4, space="PSUM"))

            h1c = _alloc(h1_pool, NM8, [P, TOWN], F32R, "h1c")

            for dc in range(DFF // CH):
                w1c = [w5.tile([P, CH], F32R, tag=f"w1c{i}",
                               name=f"w1c{dc}_{i}", bufs=2)
                       for i in range(KT_C)]
                w2c = [w5.tile([P, C], F32R, tag=f"w2c{i}",
                               name=f"w2c{dc}_{i}", bufs=2)
                       for i in range(NM8)]
                for kt in range(KT_C):
                    nc.sync.dma_start(
                        out=w1c[kt],
                        in_=fc1_w[kt * P:(kt + 1) * P,
                                  dc * CH:(dc + 1) * CH].bitcast(F32R))
                for k8 in range(NM8):
                    nc.sync.dma_start(
                        out=w2c[k8],
                        in_=fc2_w[dc * CH + k8 * P:
                                  dc * CH + (k8 + 1) * P, :].bitcast(F32R))
                for m8 in range(NM8):
                    pss = [mm_ps5.tile([P, 512], F32, tag=f"mm{nb}",
                                       name=f"f1{dc}_{m8}_{nb}")
                           for nb in range(NB_OWN)]
                    for kt in range(KT_C):
                        for nb in range(NB_OWN):
                            nc.tensor.matmul(
                                pss[nb], w1c[kt][:, m8 * P:(m8 + 1) * P],
                                h2T[kt][:, nb * 512:(nb + 1) * 512],
                                start=(kt == 0), stop=(kt == KT_C - 1))
                    for nb in range(NB_OWN):
                        nc.scalar.activation(
                            out=h1c[m8][:, nb * 512:(nb + 1) * 512],
                            in_=pss[nb], func=Relu,
                            bias=fc1b_t[:, dc * NM8 + m8:dc * NM8 + m8 + 1],
                            scale=1.0)
                for ft in range(FT_C):
                    pss = [mm_ps5.tile([P, 512], F32, tag=f"mm{nb}",
                                       name=f"f2{dc}_{ft}_{nb}")
                           for nb in range(NB_OWN)]
                    for k8 in range(NM8):
                        for nb in range(NB_OWN):
                            nc.tensor.matmul(
                                pss[nb], w2c[k8][:, ft * P:(ft + 1) * P],
                                h1c[k8][:, nb * 512:(nb + 1) * 512],
                                start=(k8 == 0), stop=(k8 == NM8 - 1))
                    for nb in range(NB_OWN):
                        sl = slice(nb * 512, (nb + 1) * 512)
                        if dc == 0:
                            nc.vector.tensor_copy(out=mlpT[ft][:, sl],
                                                  in_=pss[nb])
                        else:
                            nc.vector.tensor_add(out=mlpT[ft][:, sl],
                                                 in0=mlpT[ft][:, sl],
                                                 in1=pss[nb])

        s45.close()

        # ---- Phase 6: final residual + fc2 bias -> out (feature-major) ----
        with ExitStack() as c6:
            opool = c6.enter_context(tc.tile_pool(name="opool", bufs=3))
            for ft in range(FT_C):
                o = opool.tile([P, TOWN], F32, tag="o", name=f"o{ft}")
                nc.vector.tensor_add(out=o, in0=x2T[ft].bitcast(F32),
                                     in1=mlpT[ft])
                nc.vector.tensor_scalar_add(out=o, in0=o,
                                            scalar1=fc2b_t[:, ft:ft + 1])
                nc.sync.dma_start(out=out[ft * P:(ft + 1) * P, :], in_=o)

    nc.compile()
    return nc


_NC_CACHE = None


def _get_nc():
    global _NC_CACHE
    if _NC_CACHE is None:
        _NC_CACHE = build_nc()
    return _NC_CACHE


_CHUNKS = {0: (0, 3), 1: (1, 2)}


def _pair_mask(m):
    # [1024, 512] -> [512, 1024]: row-block k2 holds [mask(2*k2) | mask(2*k2+1)]
    return np.ascontiguousarray(
        m.reshape(4, 2, 128, 512).transpose(0, 2, 1, 3).reshape(512, 1024))


def _make_masks(cl, ch):
    k = np.arange(1024, dtype=np.int64)[:, None]
    q = np.arange(512, dtype=np.int64)[None, :]
    m_lo = np.where(k <= cl * 512 + q, 0.0, NEG).astype(np.float32)
    m_hi = np.where(1024 + k <= ch * 512 + q, 0.0, NEG).astype(np.float32)
    return _pair_mask(m_lo), _pair_mask(m_hi)


def _run(inputs, trace=False):
    nc = _get_nc()
    xs = {k: np.ascontiguousarray(np.asarray(v), dtype=np.float32)
          for k, v in inputs.items()}
    x = xs["x"]
    xT = {b: np.ascontiguousarray(x[b].T) for b in range(B)}
    in_maps = []
    for c in range(8):
        b, j = divmod(c, 2)
        cl, ch = _CHUNKS[j]
        m_lo, m_hi = _make_masks(cl, ch)
        xT_own = np.ascontiguousarray(
            np.concatenate([xT[b][:, cl * 512:(cl + 1) * 512],
                            xT[b][:, ch * 512:(ch + 1) * 512]], axis=1))
        in_maps.append({
            "xT_full": xT[b],
            "xT_own": xT_own,
            "mask_lo": m_lo,
            "mask_hi": m_hi,
            "attn_w": xs["attn_w"], "attn_b": xs["attn_b"],
            "proj_w": xs["proj_w"], "proj_b": xs["proj_b"],
            "ln1_g": xs["ln1_g"], "ln1_b": xs["ln1_b"],
            "ln2_g": xs["ln2_g"], "ln2_b": xs["ln2_b"],
            "fc1_w": xs["fc1_w"], "fc1_b": xs["fc1_b"],
            "fc2_w": xs["fc2_w"], "fc2_b": xs["fc2_b"],
        })
    res = run_bass_kernel_spmd(nc, in_maps, list(range(8)), trace=trace)
    full = np.empty((B, T, C), dtype=np.float32)
    for c in range(8):
        b, j = divmod(c, 2)
        cl, ch = _CHUNKS[j]
        o = res.results[c]["out"]            # [C, TOWN] feature-major
        full[b, cl * 512:(cl + 1) * 512] = o[:, 0:512].T
        full[b, ch * 512:(ch + 1) * 512] = o[:, 512:1024].T
    return full, res.exec_time_ns


def kernel(**inputs):
    out, _ = _run(inputs, trace=False)
    return out



# revision 16
# speedup vs baseline: 1.5084x; 1.5084x over previous
"""Trainium2 Bass kernel for a dense transformer block (nn_Block_30262339567972).

Full inputs in, full outputs out. Sharded across 8 NeuronCores with zero
collectives: core c = 2*b + j owns two 512-token chunks of batch b
(j=0 -> chunks {0,3}, j=1 -> chunks {1,2}; the pairing balances causal
attention work). The HOST PERMUTES the token axis per core to
[own_lo, own_hi, other_a, other_b] so one uniform program serves both
chunk assignments: own tokens are always columns 0-1023, and all causal
structure lives in host-computed mask data + a fixed block skip/narrow
pattern that is identical for both variants.

Everything on device is feature-major and bf16 (fp32 PSUM accumulation);
LN gains/biases are folded into the following matmul's weights on the
host, so LayerNorm on device is stats (PE ones-matmuls) + (x-mu)*rs only.
K/V/Q stay in SBUF (no DRAM spill). Attention emission is software-
pipelined (scores[i+1] lands between scores[i] and y[i] on the PE queue)
so the PE never sees a multi-microsecond idle window (keeps the HAM
clock gate at 8/8). SBUF is tight, so phases alias dead buffers:
attnT overwrites qT group-by-group, x2/h2 carve out of the dead V tiles,
and the MLP intermediate h1 reuses kT's space.
"""

from contextlib import ExitStack

import numpy as np
import ml_dtypes

import concourse.bacc as bacc
import concourse.bass as bass
import concourse.tile as tile
from concourse import mybir
from concourse.bass_utils import run_bass_kernel_spmd
import concourse.bass_utils as _bu

# NOTE: ldw-opt stays at the default (false) — walrus rejects some of this
# kernel's Ldweights under --enable-ldw-opt=true (strided V-tile weights).

F32 = mybir.dt.float32
BF16 = mybir.dt.bfloat16
P = 128
B, T, C = 4, 2048, 1024
H, D = 16, 64
DFF = 4096
TOWN = 1024
KT = C // P            # 8 feature tiles
G = H // 2             # 8 head-pair groups
NB = T // 512          # 4 token blocks of 512
EPS = 1e-5
SCALE = D ** -0.5
NEG = -1e30
BF = ml_dtypes.bfloat16
DEBUG = False

Exp = mybir.ActivationFunctionType.Exp
Relu = mybir.ActivationFunctionType.Relu
Sqrt = mybir.ActivationFunctionType.Sqrt
ADD = mybir.AluOpType.add
MULT = mybir.AluOpType.mult

# Attention pair schedule, uniform across cores (permuted kv order).
# Each entry: (ktA, ktB, qsA, qsB, mask_idx)
#   kt: kv 128-token tile in permuted order
#   qs: first query column computed for that kt (diagonal narrowing)
#   mask_idx: row in the masks input, or None for never-masked pairs
PAIR_DEFS = {
    0: [
        (0, 1, 0, 128, 0),      # own_lo diagonal
        (2, 3, 256, 384, 1),
        (8, 9, 0, 0, 2),        # other_a: data mask (all-0 or all-NEG)
        (10, 11, 0, 0, 3),
    ],
    1: [
        (0, 1, 0, 0, None),     # own_lo always fully visible from own_hi
        (2, 3, 0, 0, None),
        (4, 5, 0, 128, 4),      # own_hi diagonal
        (6, 7, 256, 384, 5),
        (8, 9, 0, 0, None),     # other_a always fully visible
        (10, 11, 0, 0, None),
        (12, 13, 0, 0, 6),      # other_b: data mask
        (14, 15, 0, 0, 7),
    ],
}


def _ln_stats(nc, src_aps, ones_bf, eps_t, stp, sqp, rowp, bcp, tag):
    """Feature-major LayerNorm stats over a 512-token block.

    src_aps: KT APs of [P, 512] bf16. Returns (mu_b, rs_b) bf16 [P, 512]
    broadcast tiles."""
    ssum = stp.tile([1, 512], F32, tag="ssum", name=f"ssum{tag}")
    ssq = stp.tile([1, 512], F32, tag="ssq", name=f"ssq{tag}")
    for kt in range(KT):
        nc.tensor.matmul(ssum, ones_bf, src_aps[kt],
                         start=(kt == 0), stop=(kt == KT - 1))
    for kt in range(KT):
        sq = sqp.tile([P, 512], BF16, tag=f"sq{kt}", name=f"sq{tag}_{kt}",
                      bufs=1)
        nc.vector.tensor_mul(out=sq, in0=src_aps[kt], in1=src_aps[kt])
        nc.tensor.matmul(ssq, ones_bf, sq,
                         start=(kt == 0), stop=(kt == KT - 1))
    mu = rowp.tile([1, 512], F32, tag="mu", name=f"mu{tag}")
    nc.vector.tensor_scalar_mul(mu, ssum, 1.0 / C)
    ms = rowp.tile([1, 512], F32, tag="ms", name=f"ms{tag}")
    nc.vector.tensor_scalar_mul(ms, ssq, 1.0 / C)
    mu2 = rowp.tile([1, 512], F32, tag="mu2", name=f"mu2{tag}")
    nc.vector.tensor_mul(out=mu2, in0=mu, in1=mu)
    nc.vector.tensor_sub(out=ms, in0=ms, in1=mu2)
    std = rowp.tile([1, 512], F32, tag="std", name=f"std{tag}")
    nc.scalar.activation(out=std, in_=ms, func=Sqrt, bias=eps_t[0:1, 0:1])
    rs = rowp.tile([1, 512], F32, tag="rs", name=f"rs{tag}")
    nc.vector.reciprocal_approx_fast(out=rs, in_=std)
    mu_bf = rowp.tile([1, 512], BF16, tag="mubf", name=f"mubf{tag}")
    nc.vector.tensor_copy(out=mu_bf, in_=mu)
    rs_bf = rowp.tile([1, 512], BF16, tag="rsbf", name=f"rsbf{tag}")
    nc.vector.tensor_copy(out=rs_bf, in_=rs)
    mu_b = bcp.tile([P, 512], BF16, tag="mub", name=f"mub{tag}")
    nc.gpsimd.partition_broadcast(mu_b, mu_bf)
    rs_b = bcp.tile([P, 512], BF16, tag="rsb", name=f"rsb{tag}")
    nc.gpsimd.partition_broadcast(rs_b, rs_bf)
    return mu_b, rs_b


def build_nc():
    nc = bacc.Bacc()
    xT = nc.declare_dram_parameter("xT", [C, T], BF16, isOutput=False)
    masks = nc.declare_dram_parameter("masks", [8, P, 1024], BF16, isOutput=False)
    attn_w = nc.declare_dram_parameter("attn_w", [C, 3 * C], BF16, isOutput=False)
    attn_b = nc.declare_dram_parameter("attn_b", [3 * C], F32, isOutput=False)
    proj_w = nc.declare_dram_parameter("proj_w", [C, C], BF16, isOutput=False)
    proj_b = nc.declare_dram_parameter("proj_b", [C], F32, isOutput=False)
    fc1_w = nc.declare_dram_parameter("fc1_w", [C, DFF], BF16, isOutput=False)
    fc1_b = nc.declare_dram_parameter("fc1_b", [DFF], F32, isOutput=False)
    fc2_w = nc.declare_dram_parameter("fc2_w", [DFF, C], BF16, isOutput=False)
    fc2_b = nc.declare_dram_parameter("fc2_b", [C], F32, isOutput=False)
    out = nc.declare_dram_parameter("out", [C, TOWN], F32, isOutput=True)
    if DEBUG:
        dbg_k = nc.declare_dram_parameter("dbg_k", [P, T], BF16, isOutput=True)
        dbg_q = nc.declare_dram_parameter("dbg_q", [P, TOWN], BF16, isOutput=True)
        dbg_v = nc.declare_dram_parameter("dbg_v", [P, 1040], BF16, isOutput=True)
        dbg_at = nc.declare_dram_parameter("dbg_at", [P, TOWN], BF16, isOutput=True)
        dbg_x2 = nc.declare_dram_parameter("dbg_x2", [P, TOWN], BF16, isOutput=True)
        dbg_h2 = nc.declare_dram_parameter("dbg_h2", [P, TOWN], BF16, isOutput=True)
        dbg_den = nc.declare_dram_parameter("dbg_den", [8, 512], F32, isOutput=True)

    with tile.TileContext(nc) as tc, ExitStack() as top:
        const = top.enter_context(tc.tile_pool(name="const", bufs=1))
        ones_bf = const.tile([P, 1], BF16, name="ones_bf")
        nc.vector.memset(ones_bf, 1.0)
        eps_t = const.tile([P, 1], F32, name="eps_t")
        nc.vector.memset(eps_t, EPS)
        abq = const.tile([P, G], F32, name="abq")
        abk = const.tile([P, G], F32, name="abk")
        nc.sync.dma_start(out=abq, in_=attn_b[0:C].rearrange("(g p) -> p g", p=P))
        nc.sync.dma_start(out=abk, in_=attn_b[C:2 * C].rearrange("(g p) -> p g", p=P))
        projb = const.tile([P, KT], F32, name="projb")
        nc.sync.dma_start(out=projb, in_=proj_b.rearrange("(f p) -> p f", p=P))
        fc2b = const.tile([P, KT], F32, name="fc2b")
        nc.sync.dma_start(out=fc2b, in_=fc2_b.rearrange("(f p) -> p f", p=P))
        fc1b = const.tile([P, DFF // P], F32, name="fc1b")
        nc.sync.dma_start(out=fc1b, in_=fc1_b.rearrange("(f p) -> p f", p=P))
        bv_bc = const.tile([P, C], F32, name="bv_bc")
        abv = attn_b[2 * C:3 * C]
        nc.sync.dma_start(
            out=bv_bc,
            in_=bass.AP(tensor=abv.tensor, offset=abv.offset,
                        ap=[[0, P]] + list(abv.ap[-1:])))

        # Long-lived activation state; later phases alias into dead regions.
        kvq = top.enter_context(tc.tile_pool(name="kvq", bufs=1))
        kT_t = [kvq.tile([P, T], BF16, tag=f"k{g}", name=f"kT{g}")
                for g in range(G)]
        qT_t = [kvq.tile([P, TOWN], BF16, tag=f"q{g}", name=f"qT{g}")
                for g in range(G)]
        vx = [kvq.tile([P, 1040], BF16, tag=f"v{tt}", name=f"v{tt}")
              for tt in range(T // P)]
        v_t = [t.rearrange("p (g x d) -> p g x d", g=G, x=2, d=65) for t in vx]
        for tt in range(T // P):
            nc.vector.memset(v_t[tt][:, :, :, 64:65], 1.0)
        # aliases (regions dead by the time they are written):
        attnT = qT_t                                   # written per-(g,hh,qc)
        x2T = [vx[2 * ft][:, 0:TOWN] for ft in range(KT)]
        h2T = [vx[2 * ft + 1][:, 0:TOWN] for ft in range(KT)]
        h1 = [kT_t[m // 2][:, (m % 2) * TOWN:(m % 2 + 1) * TOWN]
              for m in range(16)]

        # ================= Phase 1: LN1 + K/V/Q (per 512-token block) ======
        with ExitStack() as c1:
            awp = c1.enter_context(tc.tile_pool(name="awp", bufs=1))
            aw = [awp.tile([P, 3 * C], BF16, tag=f"aw{kt}", name=f"aw{kt}")
                  for kt in range(KT)]
            for kt in range(KT):
                nc.sync.dma_start(out=aw[kt], in_=attn_w[kt * P:(kt + 1) * P, :])

            xtr = c1.enter_context(tc.tile_pool(name="xtr", bufs=1))
            hp = c1.enter_context(tc.tile_pool(name="hp", bufs=2))
            sqp = c1.enter_context(tc.tile_pool(name="sqp", bufs=1))
            stp = c1.enter_context(tc.tile_pool(name="stp", bufs=1, space="PSUM"))
            rowp = c1.enter_context(tc.tile_pool(name="rowp", bufs=1))
            bcp = c1.enter_context(tc.tile_pool(name="bcp", bufs=2))
            mmp = c1.enter_context(tc.tile_pool(name="mmp", bufs=2, space="PSUM"))

            # stage all x DMAs up front (double-buffered by nb parity)
            xblk = []
            for nb in range(NB):
                tiles = []
                for kt in range(KT):
                    dst = xtr.tile([P, 512], BF16, tag=f"xt{kt}_{nb % 2}",
                                   name=f"xt{kt}_{nb}", bufs=1)
                    nc.sync.dma_start(
                        out=dst, in_=xT[kt * P:(kt + 1) * P,
                                        nb * 512:(nb + 1) * 512])
                    tiles.append(dst)
                xblk.append(tiles)

            for nb in range(NB):
                xb = xblk[nb]
                mu_b, rs_b = _ln_stats(nc, xb, ones_bf, eps_t, stp, sqp,
                                       rowp, bcp, f"a{nb}")
                hT = []
                for kt in range(KT):
                    t = hp.tile([P, 512], BF16, tag=f"hd{kt}",
                                name=f"hd{nb}_{kt}", bufs=2)
                    nc.vector.tensor_sub(out=t, in0=xb[kt], in1=mu_b)
                    nc.vector.tensor_mul(out=t, in0=t, in1=rs_b)
                    hT.append(t)

                # --- K (all groups), Q (own blocks) ---
                for g in range(G):
                    kps = mmp.tile([P, 512], F32, tag="kq", name=f"kps{nb}_{g}")
                    for kt in range(KT):
                        nc.tensor.matmul(
                            kps, aw[kt][:, C + g * P:C + (g + 1) * P], hT[kt],
                            start=(kt == 0), stop=(kt == KT - 1))
                    nc.vector.tensor_scalar_add(
                        out=kT_t[g][:, nb * 512:(nb + 1) * 512], in0=kps,
                        scalar1=abk[:, g:g + 1])
                if nb < 2:
                    for g in range(G):
                        qps = mmp.tile([P, 512], F32, tag="kq",
                                       name=f"qps{nb}_{g}")
                        for kt in range(KT):
                            nc.tensor.matmul(
                                qps, aw[kt][:, g * P:(g + 1) * P], hT[kt],
                                start=(kt == 0), stop=(kt == KT - 1))
                        nc.vector.tensor_scalar_add(
                            out=qT_t[g][:, nb * 512:(nb + 1) * 512], in0=qps,
                            scalar1=abq[:, g:g + 1])

                # --- V for this block's 4 token tiles ---
                for t4 in range(4):
                    tt = nb * 4 + t4
                    vps = [mmp.tile([P, 512], F32, tag=f"v{h_}",
                                    name=f"vps{tt}_{h_}") for h_ in range(2)]
                    for kt in range(KT):
                        for h_ in range(2):
                            nc.tensor.matmul(
                                vps[h_], hT[kt][:, t4 * P:(t4 + 1) * P],
                                aw[kt][:, 2 * C + h_ * 512:2 * C + (h_ + 1) * 512],
                                start=(kt == 0), stop=(kt == KT - 1))
                    for h_ in range(2):
                        nc.vector.tensor_add(
                            out=v_t[tt][:, 4 * h_:4 * (h_ + 1), :, 0:64],
                            in0=vps[h_].rearrange("p (g x d) -> p g x d",
                                                  x=2, d=64),
                            in1=bv_bc[:, h_ * 512:(h_ + 1) * 512].rearrange(
                                "p (g x d) -> p g x d", x=2, d=64))

        if DEBUG:
            nc.sync.dma_start(out=dbg_k[0:P, :], in_=kT_t[0])
            nc.sync.dma_start(out=dbg_q[0:P, :], in_=qT_t[0])
            nc.sync.dma_start(out=dbg_v[0:P, :], in_=vx[0])

        # ================= Phase 2: attention ==============================
        with ExitStack() as cb:
            pwp = cb.enter_context(tc.tile_pool(name="pwp", bufs=1))
            pw = [pwp.tile([P, C], BF16, tag=f"pw{kt}", name=f"pw{kt}")
                  for kt in range(KT)]
            for kt in range(KT):
                nc.sync.dma_start(out=pw[kt], in_=proj_w[kt * P:(kt + 1) * P, :])

            with ExitStack() as c2:
                mkp = c2.enter_context(tc.tile_pool(name="mkp", bufs=1))
                mk = [mkp.tile([P, 1024], BF16, tag=f"mk{i}", name=f"mk{i}")
                      for i in range(8)]
                for i in range(8):
                    nc.sync.dma_start(out=mk[i], in_=masks[i])
                scp = c2.enter_context(tc.tile_pool(name="scp", bufs=3,
                                                    space="PSUM"))
                yp = c2.enter_context(tc.tile_pool(name="yp", bufs=2,
                                                   space="PSUM"))
                ptp = c2.enter_context(tc.tile_pool(name="ptp", bufs=3))
                rcp = c2.enter_context(tc.tile_pool(name="rcp", bufs=2))
                rbp = c2.enter_context(tc.tile_pool(name="rbp", bufs=2))

                # software-pipelined emission: scores(i+1) lands on the PE
                # queue between scores(i) and y(i) so the PE never waits a
                # full mask+exp latency. Normalize is emitted right after a
                # y-group's last matmul; the pipeline flows across qc/hh/g.
                pend = [None]

                def norm(y_t, g, hh, qc):
                    hsl = slice(64 * hh, 64 * (hh + 1))
                    den = rcp.tile([1, 512], F32, tag="den",
                                   name=f"den{g}_{hh}_{qc}")
                    nc.vector.tensor_copy(out=den, in_=y_t[64:65, :])
                    rc = rcp.tile([1, 512], F32, tag="rc",
                                  name=f"rc{g}_{hh}_{qc}")
                    nc.vector.reciprocal_approx_fast(out=rc, in_=den)
                    if DEBUG and g == 0 and hh == 0:
                        nc.sync.dma_start(out=dbg_den[2 * qc:2 * qc + 1, :],
                                          in_=den)
                        nc.sync.dma_start(out=dbg_den[2 * qc + 1:2 * qc + 2, :],
                                          in_=rc)
                    rb = rbp.tile([64, 512], F32, tag="rb",
                                  name=f"rb{g}_{hh}_{qc}")
                    nc.gpsimd.partition_broadcast(rb, rc)
                    nc.vector.tensor_mul(
                        out=attnT[g][hsl, qc * 512:(qc + 1) * 512],
                        in0=y_t[0:64, :], in1=rb)

                def flush_y():
                    if pend[0] is None:
                        return
                    pts, items, y_t, first, last, g_, hh_, qc_ = pend[0]
                    for idx, (kt, off, qs, ap) in enumerate(items):
                        nc.tensor.matmul(
                            y_t[:, qs:qs + ap], v_t[kt][:, g_, hh_, :],
                            pts[:, off:off + ap],
                            start=(first and idx == 0),
                            stop=(last and idx == len(items) - 1))
                    if last:
                        norm(y_t, g_, hh_, qc_)
                    pend[0] = None

                for g in range(G):
                    for hh in range(2):
                        hsl = slice(64 * hh, 64 * (hh + 1))
                        for qc in (0, 1):
                            pairs = PAIR_DEFS[qc]
                            y_t = yp.tile([65, 512], F32, tag="y",
                                          name=f"y{g}_{hh}_{qc}")
                            for pi, (ktA, ktB, qsA, qsB, mi) in enumerate(pairs):
                                items = []
                                off = 0
                                for kt, qs in ((ktA, qsA), (ktB, qsB)):
                                    items.append((kt, off, qs, 512 - qs))
                                    off += 512 - qs
                                w = off
                                scs = scp.tile([P, 1024], F32, tag="sc",
                                               name=f"sc{g}_{hh}_{qc}_{pi}")
                                for (kt, o_, qs, ap) in items:
                                    nc.tensor.matmul(
                                        scs[:, o_:o_ + ap],
                                        kT_t[g][hsl, kt * P:(kt + 1) * P],
                                        qT_t[g][hsl,
                                                qc * 512 + qs:(qc + 1) * 512],
                                        start=True, stop=True,
                                        tile_position=(64 * hh, 0))
                                flush_y()
                                if mi is not None:
                                    nc.vector.scalar_tensor_tensor(
                                        out=scs[:, 0:w], in0=scs[:, 0:w],
                                        scalar=1.0, in1=mk[mi][:, 0:w],
                                        op0=MULT, op1=ADD)
                                pts = ptp.tile([P, 1024], BF16, tag="pt",
                                               name=f"pt{g}_{hh}_{qc}_{pi}")
                                nc.scalar.activation(out=pts[:, 0:w],
                                                     in_=scs[:, 0:w],
                                                     func=Exp, scale=SCALE)
                                pend[0] = (pts, items, y_t, pi == 0,
                                           pi == len(pairs) - 1, g, hh, qc)
                flush_y()

            if DEBUG:
                nc.sync.dma_start(out=dbg_at[0:P, :], in_=attnT[0])

            # ============= Phase 3: proj + residual + LN2 ==================
            with ExitStack() as c3:
                xo2p = c3.enter_context(tc.tile_pool(name="xo2", bufs=1))
                x_own = [xo2p.tile([P, TOWN], BF16, tag=f"xo{kt}",
                                   name=f"xo{kt}") for kt in range(KT)]
                for kt in range(KT):
                    nc.sync.dma_start(out=x_own[kt],
                                      in_=xT[kt * P:(kt + 1) * P, 0:TOWN])
                prp = c3.enter_context(tc.tile_pool(name="prp", bufs=2,
                                                    space="PSUM"))
                for ft in range(KT):
                    pp = prp.tile([P, TOWN], F32, tag="pp", name=f"pp{ft}")
                    for kt in range(KT):
                        for nbq in range(2):
                            nc.tensor.matmul(
                                pp[:, nbq * 512:(nbq + 1) * 512],
                                pw[kt][:, ft * P:(ft + 1) * P],
                                attnT[kt][:, nbq * 512:(nbq + 1) * 512],
                                start=(kt == 0), stop=(kt == KT - 1))
                    nc.vector.scalar_tensor_tensor(
                        out=x2T[ft], in0=pp, scalar=projb[:, ft:ft + 1],
                        in1=x_own[ft], op0=ADD, op1=ADD)

                # LN2 over the two own blocks
                stp2 = c3.enter_context(tc.tile_pool(name="stp2", bufs=1,
                                                     space="PSUM"))
                sqp2 = c3.enter_context(tc.tile_pool(name="sqp2", bufs=1))
                rowp2 = c3.enter_context(tc.tile_pool(name="rowp2", bufs=1))
                bcp2 = c3.enter_context(tc.tile_pool(name="bcp2", bufs=2))
                for nb in range(2):
                    sl = slice(nb * 512, (nb + 1) * 512)
                    mu_b, rs_b = _ln_stats(
                        nc, [x2T[kt][:, sl] for kt in range(KT)], ones_bf,
                        eps_t, stp2, sqp2, rowp2, bcp2, f"b{nb}")
                    for kt in range(KT):
                        nc.vector.tensor_sub(out=h2T[kt][:, sl],
                                             in0=x2T[kt][:, sl], in1=mu_b)
                        nc.vector.tensor_mul(out=h2T[kt][:, sl],
                                             in0=h2T[kt][:, sl], in1=rs_b)

        if DEBUG:
            nc.sync.dma_start(out=dbg_x2[0:P, :], in_=x2T[0])
            nc.sync.dma_start(out=dbg_h2[0:P, :], in_=h2T[0])

        # ================= Phase 4: MLP (2 chunks of 2048 dff) =============
        with ExitStack() as c4:
            w1p = c4.enter_context(tc.tile_pool(name="w1p", bufs=1))
            w2p = c4.enter_context(tc.tile_pool(name="w2p", bufs=1))
            accp = c4.enter_context(tc.tile_pool(name="accp", bufs=1))
            outp = c4.enter_context(tc.tile_pool(name="outp", bufs=3))
            f1p = c4.enter_context(tc.tile_pool(name="f1p", bufs=2, space="PSUM"))
            f2p = c4.enter_context(tc.tile_pool(name="f2p", bufs=2, space="PSUM"))
            acc = [accp.tile([P, TOWN], F32, tag=f"ac{ft}", name=f"acc{ft}")
                   for ft in range(KT)]

            for dc in range(2):
                w1 = [w1p.tile([P, 2048], BF16, tag=f"w1_{kt}",
                               name=f"w1_{dc}_{kt}", bufs=1)
                      for kt in range(KT)]
                for kt in range(KT):
                    nc.sync.dma_start(
                        out=w1[kt],
                        in_=fc1_w[kt * P:(kt + 1) * P,
                                  dc * 2048:(dc + 1) * 2048])
                w2 = [w2p.tile([P, C], BF16, tag=f"w2_{m}",
                               name=f"w2_{dc}_{m}", bufs=1)
                      for m in range(16)]
                for m in range(16):
                    nc.sync.dma_start(
                        out=w2[m],
                        in_=fc2_w[dc * 2048 + m * P:dc * 2048 + (m + 1) * P, :])
                for m in range(16):
                    f1 = f1p.tile([P, TOWN], F32, tag="f1", name=f"f1_{dc}_{m}")
                    for kt in range(KT):
                        for nbq in range(2):
                            nc.tensor.matmul(
                                f1[:, nbq * 512:(nbq + 1) * 512],
                                w1[kt][:, m * P:(m + 1) * P],
                                h2T[kt][:, nbq * 512:(nbq + 1) * 512],
                                start=(kt == 0), stop=(kt == KT - 1))
                    nc.scalar.activation(
                        out=h1[m], in_=f1, func=Relu,
                        bias=fc1b[:, dc * 16 + m:dc * 16 + m + 1])
                for ft in range(KT):
                    f2 = f2p.tile([P, TOWN], F32, tag="f2", name=f"f2_{dc}_{ft}")
                    for m in range(16):
                        for nbq in range(2):
                            nc.tensor.matmul(
                                f2[:, nbq * 512:(nbq + 1) * 512],
                                w2[m][:, ft * P:(ft + 1) * P],
                                h1[m][:, nbq * 512:(nbq + 1) * 512],
                                start=(m == 0), stop=(m == 15))
                    if dc == 0:
                        nc.vector.scalar_tensor_tensor(
                            out=acc[ft], in0=f2, scalar=fc2b[:, ft:ft + 1],
                            in1=x2T[ft], op0=ADD, op1=ADD)
                    else:
                        o = outp.tile([P, TOWN], F32, tag="o", name=f"o{ft}")
                        nc.vector.tensor_add(out=o, in0=f2, in1=acc[ft])
                        nc.sync.dma_start(out=out[ft * P:(ft + 1) * P, :], in_=o)

    nc.compile()
    return nc


_NC_CACHE = None


def _get_nc():
    global _NC_CACHE
    if _NC_CACHE is None:
        _NC_CACHE = build_nc()
    return _NC_CACHE


_CHUNKS = {0: (0, 3), 1: (1, 2)}


def _perm_chunks(j):
    cl, ch = _CHUNKS[j]
    others = [c for c in range(4) if c not in (cl, ch)]
    return [cl, ch] + others


def _make_masks(perm):
    """[8, 128, 1024] bf16 per PAIR_DEFS packing, in permuted kv order."""
    kv_tok = np.concatenate([np.arange(c * 512, (c + 1) * 512) for c in perm])
    out = np.zeros((8, P, 1024), dtype=np.float32)
    for qc in (0, 1):
        q_tok = kv_tok[qc * 512:(qc + 1) * 512]
        for (ktA, ktB, qsA, qsB, mi) in PAIR_DEFS[qc]:
            if mi is None:
                continue
            off = 0
            for kt, qs in ((ktA, qsA), (ktB, qsB)):
                w = 512 - qs
                kvg = kv_tok[kt * P:(kt + 1) * P][:, None]
                qg = q_tok[None, qs:512]
                out[mi, :, off:off + w] = np.where(kvg <= qg, 0.0, NEG)
                off += w
    return out.astype(BF)


def _run(inputs, trace=False):
    nc = _get_nc()
    xs = {k: np.asarray(v, dtype=np.float32) for k, v in inputs.items()}
    # fold LN gains/biases into the following matmuls (host-side)
    attn_w = xs["ln1_g"][:, None] * xs["attn_w"]
    attn_b = xs["attn_b"] + xs["ln1_b"] @ xs["attn_w"]
    fc1_w = xs["ln2_g"][:, None] * xs["fc1_w"]
    fc1_b = xs["fc1_b"] + xs["ln2_b"] @ xs["fc1_w"]
    wcast = {
        "attn_w": np.ascontiguousarray(attn_w).astype(BF), "attn_b": attn_b,
        "proj_w": np.ascontiguousarray(xs["proj_w"]).astype(BF),
        "proj_b": xs["proj_b"],
        "fc1_w": np.ascontiguousarray(fc1_w).astype(BF), "fc1_b": fc1_b,
        "fc2_w": np.ascontiguousarray(xs["fc2_w"]).astype(BF),
        "fc2_b": xs["fc2_b"],
    }
    x = xs["x"]
    in_maps = []
    for c in range(8):
        b, j = divmod(c, 2)
        perm = _perm_chunks(j)
        tok = np.concatenate([np.arange(cc * 512, (cc + 1) * 512)
                              for cc in perm])
        xTh = np.ascontiguousarray(x[b].T[:, tok]).astype(BF)
        in_maps.append({"xT": xTh, "masks": _make_masks(perm), **wcast})
    res = run_bass_kernel_spmd(nc, in_maps, list(range(8)), trace=trace)
    full = np.empty((B, T, C), dtype=np.float32)
    for c in range(8):
        b, j = divmod(c, 2)
        cl, ch = _CHUNKS[j]
        o = res.results[c]["out"]            # [C, TOWN] feature-major
        full[b, cl * 512:(cl + 1) * 512] = o[:, 0:512].T
        full[b, ch * 512:(ch + 1) * 512] = o[:, 512:1024].T
    return full, res.exec_time_ns


def kernel(**inputs):
    out, _ = _run(inputs, trace=False)
    return out
